# revision 1
# baseline (speedup 1.0000x reference)
"""Conv2d(128->256, 3x3, stride 1, pad 1) on (32,128,56,56) fp32, data-parallel over 8 NeuronCores.

Strategy per core (4 images):
  - Host pads x spatially to 58x58 so every conv window read is a clean strided
    SBUF access; host pre-transposes weight to [I=128, 9, O=256] so all DMAs are
    contiguous per partition.
  - Conv = 9 accumulating matmuls per output tile: out[o, h*56+w] += sum_i
    w[i, kh*3+kw, o] * xpad[i, (h+kh)*58 + (w+kw)].  K = I = 128 (partition dim),
    M = 128 (half of O=256), N = 448 (8 output rows x 56 cols, one PSUM bank).
  - x and weight are declared float32r end-to-end (bit-identical to fp32 on
    the host): the PE reads 4-byte fp32, truncates to FP22, and runs at
    1 cycle/row for N>=256 (4x faster than true fp32; ~1e-4 rel err here).
  - PSUM -> SBUF copy fuses the bias add (ScalarE/VectorE alternating), then
    contiguous DMA to DRAM.
"""

import numpy as np

import concourse.bass as bass  # noqa: F401  (AP types come through bacc)
import concourse.mybir as mybir
import concourse.tile as tile
from concourse import bacc
from concourse.bass_utils import run_bass_kernel_spmd

N_CORES = 8
N_IMG = 4  # images per core
C_IN = 128
C_OUT = 256
H = W = 56
HP = WP = 58
SP = HP * WP  # 3364 padded spatial
SO = H * W  # 3136 output spatial
NROW = 8  # output rows per PSUM chunk
NCH = NROW * W  # 448 columns per matmul
RCHUNKS = H // NROW  # 7

_CACHE = {}


def _build_module(rchunks=RCHUNKS, repeat=1):
    nc = bacc.Bacc("TRN2", target_bir_lowering=False, debug=False)

    f32 = mybir.dt.float32
    f32r = mybir.dt.float32r

    xp = nc.dram_tensor("xp", [N_IMG, C_IN, SP], f32r, kind="ExternalInput").ap()
    wt = nc.dram_tensor("wt", [C_IN, 9 * C_OUT], f32r, kind="ExternalInput").ap()
    br = nc.dram_tensor("br", [C_IN, 2], f32, kind="ExternalInput").ap()
    out = nc.dram_tensor("out", [N_IMG, C_OUT, SO], f32, kind="ExternalOutput").ap()

    with tile.TileContext(nc) as tc:
        with (
            tc.tile_pool(name="const", bufs=1) as cpool,
            tc.tile_pool(name="xin", bufs=2 if repeat > 1 else 1) as xpool,
            tc.tile_pool(name="osb", bufs=3) as opool,
            tc.tile_pool(name="pp", bufs=8, space="PSUM") as ppool,
        ):
            w_sb = cpool.tile([C_IN, 9, C_OUT], f32r)
            b_sb = cpool.tile([C_IN, 2], f32)
            wt_v = wt.rearrange("i (k o) -> i k o", k=9)

            # PE clock warmup: the HAM throttles the PE to half clock until
            # ~3.4us of sustained activity. Burn dummy matmuls on a zeroed
            # scratch tile while the first input DMAs are still in flight, so
            # the real matmul stream starts at full clock.
            WARM_N = 64
            warm_sb = cpool.tile([C_IN, WARM_N], f32r)
            # (memset cannot emit float32r at the ISA level, but a DVE copy
            # with f32r output can -- zero a f32 tile and convert)
            warm_f32 = cpool.tile([C_IN, WARM_N], f32)
            nc.vector.memset(warm_f32, 0.0)
            nc.vector.tensor_copy(warm_sb, warm_f32)
            ps_warm = ppool.tile([128, NCH], f32, tag="ps")
            N_WARM = 40
            for i in range(N_WARM):
                nc.tensor.matmul(
                    ps_warm[:WARM_N, :WARM_N],
                    lhsT=warm_sb[:, :WARM_N],
                    rhs=warm_sb,
                    start=(i == 0),
                    stop=(i == N_WARM - 1),
                )

            H_BANDS = [(0, 16), (16, 30), (30, 44), (44, HP)]

            def load_image(x_sb, n, first=False):
                if first:
                    # Head-critical pieces go FIRST on the SP sequencer so
                    # they are first in line at the shared DMA engines: rows
                    # 0-9 of image 0, then the o2=0 weight taps in
                    # consumption order. Everything else follows on gpsimd.
                    nc.sync.dma_start(out=x_sb[:, n, : 10 * WP], in_=xp[n, :, : 10 * WP])
                    for k0 in (0, 3, 6):
                        nc.sync.dma_start(
                            out=w_sb[:, k0 : k0 + 3, 0:128], in_=wt_v[:, k0 : k0 + 3, 0:128]
                        )
                    nc.gpsimd.dma_start(out=b_sb, in_=br)
                    nc.gpsimd.dma_start(
                        out=x_sb[:, n, 10 * WP : 24 * WP], in_=xp[n, :, 10 * WP : 24 * WP]
                    )
                    nc.gpsimd.dma_start(
                        out=x_sb[:, n, 24 * WP : 40 * WP], in_=xp[n, :, 24 * WP : 40 * WP]
                    )
                    nc.gpsimd.dma_start(out=x_sb[:, n, 40 * WP :], in_=xp[n, :, 40 * WP :])
                    nc.gpsimd.dma_start(out=w_sb[:, :, 128:256], in_=wt_v[:, :, 128:256])
                else:
                    for h0, h1 in H_BANDS:
                        nc.sync.dma_start(
                            out=x_sb[:, n, h0 * WP : h1 * WP],
                            in_=xp[n, :, h0 * WP : h1 * WP],
                        )

            first_rep = True
            for _rep in range(repeat):
                x_sb = xpool.tile([C_IN, N_IMG, SP], f32r, tag="x_sb")
                load_image(x_sb, 0, first=first_rep)
                first_rep = False
                # [C_IN, N_IMG, HP, WP] view for conv-window slicing
                x_v = x_sb.rearrange("c n (h w) -> c n h w", h=HP)

                for n in range(N_IMG):
                    if n + 1 < N_IMG:
                        # prefetch the next image one compute-block ahead
                        load_image(x_sb, n + 1)
                    for o2 in range(2):
                        o_sb = opool.tile([128, SO], f32, tag="o_sb")
                        for r in range(rchunks):
                            ps = ppool.tile([128, NCH], f32, tag="ps")
                            lhs_base = w_sb[:, :, o2 * 128 : (o2 + 1) * 128]
                            for kh in range(3):
                                for kw in range(3):
                                    k = kh * 3 + kw
                                    rhs = x_v[:, n, r * NROW + kh : r * NROW + kh + NROW, kw : kw + W]
                                    nc.tensor.matmul(
                                        ps,
                                        lhsT=lhs_base[:, k],
                                        rhs=rhs,
                                        start=(k == 0),
                                        stop=(k == 8),
                                    )
                            dst = o_sb[:, r * NCH : (r + 1) * NCH]
                            bias_ap = b_sb[:, o2 : o2 + 1]
                            o_slice = out[n, o2 * 128 : (o2 + 1) * 128, r * NCH : (r + 1) * NCH]
                            is_last = n == N_IMG - 1 and o2 == 1 and r == rchunks - 1
                            if is_last:
                                # tail chunk: halve the copy across both engines
                                # and split the store so the exit drain starts
                                # sooner
                                hc = NCH // 2
                                nc.vector.tensor_scalar_add(dst[:, :hc], ps[:, :hc], bias_ap)
                                nc.scalar.activation(
                                    dst[:, hc:],
                                    ps[:, hc:],
                                    mybir.ActivationFunctionType.Identity,
                                    bias=bias_ap,
                                )
                                nc.sync.dma_start(out=o_slice[:, :hc], in_=dst[:, :hc])
                                nc.sync.dma_start(out=o_slice[:, hc:], in_=dst[:, hc:])
                            else:
                                if r % 2 == 0:
                                    nc.vector.tensor_scalar_add(dst, ps, bias_ap)
                                else:
                                    nc.scalar.activation(
                                        dst, ps, mybir.ActivationFunctionType.Identity, bias=bias_ap
                                    )
                                nc.sync.dma_start(out=o_slice, in_=dst)

    nc.compile()
    return nc


def _get_module():
    if "nc" not in _CACHE:
        _CACHE["nc"] = _build_module()
    return _CACHE["nc"]


def kernel(x, weight, bias):
    x = np.asarray(x, dtype=np.float32)
    weight = np.asarray(weight, dtype=np.float32)
    bias = np.asarray(bias, dtype=np.float32)

    xp = np.pad(x, ((0, 0), (0, 0), (1, 1), (1, 1))).reshape(32, C_IN, SP)
    wt = np.ascontiguousarray(weight.transpose(1, 2, 3, 0)).reshape(C_IN, 9 * C_OUT)
    br = np.ascontiguousarray(bias.reshape(2, 128).T)

    nc = _get_module()
    in_maps = [
        {"xp": np.ascontiguousarray(xp[N_IMG * c : N_IMG * (c + 1)]), "wt": wt, "br": br}
        for c in range(N_CORES)
    ]
    res = run_bass_kernel_spmd(nc, in_maps, core_ids=list(range(N_CORES)))
    outs = [r["out"].reshape(N_IMG, C_OUT, H, W) for r in res.results]
    return np.concatenate(outs, axis=0)



# revision 8
# speedup vs baseline: 1.4039x; 1.4039x over previous
"""Conv2d(128->256, 3x3, stride 1, pad 1) on (32,128,56,56) fp32, data-parallel
over 8 NeuronCores, computed in fp8e4 (e4m3) with DoubleRow matmuls.

Per core (4 images):
  - Host splits x and w into fp8 hi + lo parts: xh = fp8(x), xl = fp8(x - xh),
    wh = fp8(w), wl = fp8(w - wh). The conv is computed as
        (xh + xl) * wh  (all 9 taps)  +  xh * wl  (taps 0..5)
    giving ~1.4e-2 rel fro error (gate 2e-2); leaving 3 taps w-uncorrected
    saves 2 matmuls per chunk.
  - DoubleRow perf mode contracts 2 k-tiles (2x128 K values) per instruction
    at 0.5 cycles/row -- 2x the bf16/f32r rate. K-tile pairs are built as
    overlapping strided SBUF views (hand-written access patterns):
      * (xh, xl) hi/lo pairs for the wh terms: k-tile stride = hi->lo offset
      * (tap t, tap t+1) pairs for the wl terms: k-tile stride = tap offset
    Weights are laid out so each pair is a plain slice: wh duplicated along a
    k-tile axis, wl with adjacent taps.
  - 12 DoubleRow matmuls per 8-row output chunk (N=448, one PSUM bank),
    7 chunks x 2 out-halves x 4 images = 672 matmuls x 93ns = 62.7us PE.
  - PSUM -> SBUF copy fuses the bias add (ScalarE/VectorE alternating), then
    contiguous DMA to DRAM (fp32).
"""

import numpy as np
import ml_dtypes

import bass_rust
import concourse.bass as bass  # noqa: F401
import concourse.mybir as mybir
import concourse.tile as tile
from concourse import bacc
from concourse.bass_utils import run_bass_kernel_spmd

N_CORES = 8
N_IMG = 4  # images per core
C_IN = 128
C_OUT = 256
H = W = 56
HP = WP = 58
SP = HP * WP  # 3364 padded spatial
SO = H * W  # 3136 output spatial
NROW = 8  # output rows per PSUM chunk
NCH = NROW * W  # 448 columns per matmul
RCHUNKS = H // NROW  # 7
NTAP = 9
# (t, t+3) tap pairs carrying the wl correction: k-tile stride 58 (one padded
# row).  NB a k-tile stride of 1 hard-crashes the PE when the matmul is not
# the first of its accumulation group, so pair taps vertically, not
# horizontally.  Corrected taps = {0..5}.
CORR_PAIRS = (0, 1, 2)
TAP_OFF = [kh * WP + kw for kh in range(3) for kw in range(3)]

F8 = mybir.dt.float8e4
NP8 = ml_dtypes.float8_e4m3

_CACHE = {}


def _sv(ap_obj, dims, extra=0):
    """Hand-built (possibly overlapping) strided view of an AP."""
    c = ap_obj.copy()
    c.ap = bass_rust.VecI64Pair([list(d) for d in dims])
    c.offset = c.offset + extra
    return c


def _build_module():
    nc = bacc.Bacc("TRN2", target_bir_lowering=False, debug=False)

    f32 = mybir.dt.float32
    f32r = mybir.dt.float32r
    DR = mybir.MatmulPerfMode.DoubleRow

    # x8: [hi/lo, img, chan, padded-spatial] fp8
    x8 = nc.dram_tensor("x8", [2, N_IMG, C_IN, SP], F8, kind="ExternalInput").ap()
    # whd: wh duplicated along a k-tile axis: [c, o2, 2, tap, 128]
    whd = nc.dram_tensor("whd", [C_IN, 2 * 2 * NTAP * 128], F8, kind="ExternalInput").ap()
    # wl: [c, o2, pair3, 2, 128] -- pair p holds (wl[tap p], wl[tap p+3])
    wl = nc.dram_tensor("wl", [C_IN, 2 * 3 * 2 * 128], F8, kind="ExternalInput").ap()
    br = nc.dram_tensor("br", [C_IN, 2], f32, kind="ExternalInput").ap()
    out = nc.dram_tensor("out", [N_IMG, C_OUT, SO], f32, kind="ExternalOutput").ap()

    whd_v = whd.rearrange("c (h u t o) -> c h u t o", h=2, u=2, t=NTAP)
    wl_v = wl.rearrange("c (h p u o) -> c h p u o", h=2, p=3, u=2)

    with tile.TileContext(nc) as tc:
        with (
            tc.tile_pool(name="const", bufs=1) as cpool,
            tc.tile_pool(name="osb", bufs=3) as opool,
            tc.tile_pool(name="pp", bufs=8, space="PSUM") as ppool,
        ):
            x_sb = cpool.tile([C_IN, 2, N_IMG, SP], F8)
            whd_sb = cpool.tile([C_IN, 2, 2, NTAP, 128], F8)
            wl_sb = cpool.tile([C_IN, 2, 3, 2, 128], F8)
            b_sb = cpool.tile([C_IN, 2], f32)

            # ---- DMA plan: head-critical pieces first on the SP queue ----
            # chunk (n=0, o2=0, r=0) needs: whd half 0, wl half 0, x img0
            # hi+lo rows 0..9.  Everything else streams behind on gpsimd.
            nc.sync.dma_start(out=whd_sb[:, 0], in_=whd_v[:, 0])
            nc.sync.dma_start(out=wl_sb[:, 0], in_=wl_v[:, 0])
            nc.sync.dma_start(out=x_sb[:, 0, 0, : 10 * WP], in_=x8[0, 0, :, : 10 * WP])
            nc.sync.dma_start(out=x_sb[:, 1, 0, : 10 * WP], in_=x8[1, 0, :, : 10 * WP])
            nc.gpsimd.dma_start(out=b_sb, in_=br)
            nc.gpsimd.dma_start(out=whd_sb[:, 1], in_=whd_v[:, 1])
            nc.gpsimd.dma_start(out=wl_sb[:, 1], in_=wl_v[:, 1])
            nc.gpsimd.dma_start(out=x_sb[:, 0, 0, 10 * WP :], in_=x8[0, 0, :, 10 * WP :])
            nc.gpsimd.dma_start(out=x_sb[:, 1, 0, 10 * WP :], in_=x8[1, 0, :, 10 * WP :])
            for n in range(1, N_IMG):
                nc.gpsimd.dma_start(out=x_sb[:, 0, n], in_=x8[0, n])
                nc.gpsimd.dma_start(out=x_sb[:, 1, n], in_=x8[1, n])

            # ---- PE clock warmup while the first DMAs are in flight ----
            # (the HAM runs the PE at reduced clock until ~3us of sustained
            # activity; burn dummy f32r matmuls on a zeroed scratch tile)
            WARM_N = 64
            warm_sb = cpool.tile([C_IN, WARM_N], f32r)
            warm_f32 = cpool.tile([C_IN, WARM_N], f32)
            nc.vector.memset(warm_f32, 0.0)
            nc.vector.tensor_copy(warm_sb, warm_f32)
            ps_warm = ppool.tile([128, NCH], f32, tag="ps")
            N_WARM = 26
            for i in range(N_WARM):
                nc.tensor.matmul(
                    ps_warm[:WARM_N, :WARM_N],
                    lhsT=warm_sb[:, :WARM_N],
                    rhs=warm_sb,
                    start=(i == 0),
                    stop=(i == N_WARM - 1),
                )

            # strides for the hand-built rhs views
            hi0 = x_sb[:, 0, 0, :]
            pstride = hi0.ap[0][0]
            d_lo = x_sb[:, 1, 0, :].offset - hi0.offset  # hi -> lo k-tile stride

            for n in range(N_IMG):
                base = x_sb[:, 0, n, :]  # hi plane of image n
                for o2 in range(2):
                    o_sb = opool.tile([128, SO], f32, tag="o_sb")
                    for r in range(RCHUNKS):
                        ps = ppool.tile([128, NCH], f32, tag="ps")
                        r0 = r * NROW * WP
                        # (xh + xl) * wh : all 9 taps, hi/lo k-tile pairs
                        for t in range(NTAP):
                            rhs = _sv(
                                base,
                                [[pstride, 128], [d_lo, 2], [WP, NROW], [1, W]],
                                extra=r0 + TAP_OFF[t],
                            )
                            nc.tensor.matmul(
                                ps,
                                lhsT=whd_sb[:, o2, :, t, :],
                                rhs=rhs,
                                start=(t == 0),
                                stop=False,
                                perf_mode=DR,
                            )
                        # xh * wl : taps (t, t+3) pairs, k-tile stride 58
                        for j, t in enumerate(CORR_PAIRS):
                            rhs = _sv(
                                base,
                                [
                                    [pstride, 128],
                                    [TAP_OFF[t + 3] - TAP_OFF[t], 2],
                                    [WP, NROW],
                                    [1, W],
                                ],
                                extra=r0 + TAP_OFF[t],
                            )
                            nc.tensor.matmul(
                                ps,
                                lhsT=wl_sb[:, o2, j, :, :],
                                rhs=rhs,
                                start=False,
                                stop=(j == len(CORR_PAIRS) - 1),
                                perf_mode=DR,
                            )

                        dst = o_sb[:, r * NCH : (r + 1) * NCH]
                        bias_ap = b_sb[:, o2 : o2 + 1]
                        o_slice = out[n, o2 * 128 : (o2 + 1) * 128, r * NCH : (r + 1) * NCH]
                        is_last = n == N_IMG - 1 and o2 == 1 and r == RCHUNKS - 1
                        if is_last:
                            # tail chunk: halve the copy across both engines
                            # and split the store so the exit drain starts
                            # sooner
                            hc = NCH // 2
                            nc.vector.tensor_scalar_add(dst[:, :hc], ps[:, :hc], bias_ap)
                            nc.scalar.activation(
                                dst[:, hc:],
                                ps[:, hc:],
                                mybir.ActivationFunctionType.Identity,
                                bias=bias_ap,
                            )
                            nc.sync.dma_start(out=o_slice[:, :hc], in_=dst[:, :hc])
                            nc.sync.dma_start(out=o_slice[:, hc:], in_=dst[:, hc:])
                        else:
                            if r % 2 == 0:
                                nc.vector.tensor_scalar_add(dst, ps, bias_ap)
                            else:
                                nc.scalar.activation(
                                    dst, ps, mybir.ActivationFunctionType.Identity, bias=bias_ap
                                )
                            nc.sync.dma_start(out=o_slice, in_=dst)

    nc.compile()
    return nc


def _get_module():
    if "nc" not in _CACHE:
        _CACHE["nc"] = _build_module()
    return _CACHE["nc"]


def kernel(x, weight, bias):
    x = np.asarray(x, dtype=np.float32)
    weight = np.asarray(weight, dtype=np.float32)
    bias = np.asarray(bias, dtype=np.float32)

    xp = np.pad(x, ((0, 0), (0, 0), (1, 1), (1, 1))).reshape(32, C_IN, SP)
    xh = xp.astype(NP8)
    xl = (xp - xh.astype(np.float32)).astype(NP8)

    # weight (O, I, 3, 3) -> [I, tap, O] fp8 hi + lo
    wt = np.ascontiguousarray(weight.transpose(1, 2, 3, 0)).reshape(C_IN, NTAP, C_OUT)
    wh = wt.astype(NP8)
    wlv = (wt - wh.astype(np.float32)).astype(NP8)
    # whd: [c, o2, dup2, tap, 128]
    wh_s = wh.reshape(C_IN, NTAP, 2, 128).transpose(0, 2, 1, 3)  # [c, o2, tap, 128]
    whd = np.ascontiguousarray(
        np.broadcast_to(wh_s[:, :, None], (C_IN, 2, 2, NTAP, 128))
    ).reshape(C_IN, -1)
    # wl: [c, o2, pair3, dup2, 128] with pair p = (tap p, tap p+3)
    wl_s = wlv.reshape(C_IN, NTAP, 2, 128).transpose(0, 2, 1, 3)  # [c, o2, tap, 128]
    wlr = np.ascontiguousarray(
        np.stack([wl_s[:, :, 0:3], wl_s[:, :, 3:6]], axis=3)  # [c, o2, 3, 2, 128]
    ).reshape(C_IN, -1)
    br = np.ascontiguousarray(bias.reshape(2, 128).T)

    nc = _get_module()
    in_maps = [
        {
            "x8": np.ascontiguousarray(
                np.stack([xh[N_IMG * c : N_IMG * (c + 1)], xl[N_IMG * c : N_IMG * (c + 1)]])
            ),
            "whd": whd,
            "wl": wlr,
            "br": br,
        }
        for c in range(N_CORES)
    ]
    res = run_bass_kernel_spmd(nc, in_maps, core_ids=list(range(N_CORES)))
    outs = [r["out"].reshape(N_IMG, C_OUT, H, W) for r in res.results]
    return np.concatenate(outs, axis=0)


# revision 17
# speedup vs baseline: 1.4362x; 1.0231x over previous
"""Conv2d(128->256, 3x3, stride 1, pad 1) on (32,128,56,56) fp32, data-parallel
over 8 NeuronCores, computed in fp8e4 (e4m3) with DoubleRow matmuls.

Per core (4 images):
  - Host splits x and w into fp8 hi + lo parts: xh = fp8(x), xl = fp8(x - xh),
    wh = fp8(w), wl = fp8(w - wh). The conv is computed as
        (xh + xl) * wh  (all 9 taps)  +  xh * wl  (taps 0..5)
    giving ~1.44e-2 rel fro error incl. the bf16 output round (gate 2e-2).
  - DoubleRow perf mode contracts 2 k-tiles (2x128 K values) per instruction
    at 0.5 cycles/row -- 2x the bf16/f32r rate.  K-tile pairs are built as
    overlapping strided SBUF views (hand-written access patterns):
      * (xh, xl) hi/lo pairs for the wh terms: k-tile stride = hi->lo offset
      * (tap t, tap t+3) pairs for the wl terms: k-tile stride = 58 (one
        padded row).  NB a k-tile stride of 1 hard-crashes the PE when the
        matmul is not first in its accumulation group, so taps pair
        vertically, never horizontally.
  - 12 DoubleRow matmuls per 8-row output chunk (N=448, one PSUM bank),
    7 chunks x 2 out-halves x 4 images = 672 matmuls x 93ns = 62.7us PE.
  - PSUM -> SBUF copy fuses the bias add (ScalarE/VectorE alternating) and
    narrows to bf16, halving the output DMA; the host widens back to fp32.
"""

import numpy as np
import ml_dtypes

import bass_rust
import concourse.bass as bass  # noqa: F401
import concourse.mybir as mybir
import concourse.tile as tile
from concourse import bacc
from concourse.bass_utils import run_bass_kernel_spmd

N_CORES = 8
N_IMG = 4  # images per core
C_IN = 128
C_OUT = 256
H = W = 56
HP = WP = 58
SP = HP * WP  # 3364 padded spatial
SO = H * W  # 3136 output spatial
NROW = 8  # output rows per PSUM chunk
NCH = NROW * W  # 448 columns per matmul
RCHUNKS = H // NROW  # 7
NTAP = 9
TAP_OFF = [kh * WP + kw for kh in range(3) for kw in range(3)]

F8 = mybir.dt.float8e4
NP8 = ml_dtypes.float8_e4m3

_CACHE = {}


def _sv(ap_obj, dims, extra=0):
    """Hand-built (possibly overlapping) strided view of an AP."""
    c = ap_obj.copy()
    c.ap = bass_rust.VecI64Pair([list(d) for d in dims])
    c.offset = c.offset + extra
    return c


def _build_module():
    nc = bacc.Bacc("TRN2", target_bir_lowering=False, debug=False)

    f32 = mybir.dt.float32
    bf16 = mybir.dt.bfloat16
    DR = mybir.MatmulPerfMode.DoubleRow

    # x8: [hi/lo, img, chan, padded-spatial] fp8
    x8 = nc.dram_tensor("x8", [2, N_IMG, C_IN, SP], F8, kind="ExternalInput").ap()
    # wts: [c, o2, slot24, 128] fp8: slots 2t,2t+1 = (wh[t], wh[t]) for the 9
    # hi/lo-pair matmuls; slots 18+2p,19+2p = (wl[p], wl[p+3]) for the 3
    # correction pairs.
    wts = nc.dram_tensor("wts", [C_IN, 2 * 24 * 128], F8, kind="ExternalInput").ap()
    br = nc.dram_tensor("br", [C_IN, 2], f32, kind="ExternalInput").ap()
    out = nc.dram_tensor("out", [N_IMG, C_OUT, SO], bf16, kind="ExternalOutput").ap()

    wts_v = wts.rearrange("c (h s o) -> c h s o", h=2, s=24)

    with tile.TileContext(nc) as tc:
        with (
            tc.tile_pool(name="const", bufs=1) as cpool,
            tc.tile_pool(name="osb", bufs=3) as opool,
            tc.tile_pool(name="pp", bufs=8, space="PSUM") as ppool,
        ):
            x_sb = cpool.tile([C_IN, 2, N_IMG, SP], F8)
            w_sb = cpool.tile([C_IN, 2, 24, 128], F8)
            b_sb = cpool.tile([C_IN, 2], f32)

            # ---- PE clock warmup: pin pe_busy_start as early as possible.
            # (the HAM p-state ramp counts from the first PE activity; a few
            # dummy f32 matmuls on a zeroed scratch tile suffice -- idle gaps
            # before the real stream do not reset the ramp.  The memzero goes
            # on Pool, which is free right after the entry barrier.)
            WARM_N = 64
            warm_sb = cpool.tile([C_IN, WARM_N], f32)
            nc.gpsimd.memzero(warm_sb)
            ps_warm = ppool.tile([128, NCH], f32, tag="ps")
            N_WARM = 8
            for i in range(N_WARM):
                nc.tensor.matmul(
                    ps_warm[:WARM_N, :WARM_N],
                    lhsT=warm_sb[:, :WARM_N],
                    rhs=warm_sb,
                    start=(i == 0),
                    stop=(i == N_WARM - 1),
                )
            # Keep the Pool DGE busy for ~3.5us so the gpsimd bulk transfers
            # below don't contend with img0's head-critical bands on the
            # shared DMA pipe.
            delay_sb = cpool.tile([C_IN, 2600], F8)
            nc.gpsimd.memzero(delay_sb)

            # ---- DMA plan: head-critical pieces first on the SP queue ----
            # chunk (n=0, o2=0, r) needs: wts half 0, x img0 hi+lo rows
            # <= 8r+9.  Stream img0 in row bands (hi+lo merged per band);
            # o2=1 weights and imgs 1-3 follow on the gpsimd queue.
            nc.sync.dma_start(out=w_sb[:, 0, 0:12], in_=wts_v[:, 0, 0:12])
            nc.sync.dma_start(out=x_sb[:, :, 0, : 10 * WP], in_=x8[:, 0, :, : 10 * WP].transpose([1, 0, 2]))
            nc.sync.dma_start(out=w_sb[:, 0, 12:24], in_=wts_v[:, 0, 12:24])
            nc.sync.dma_start(
                out=x_sb[:, :, 0, 10 * WP : 18 * WP],
                in_=x8[:, 0, :, 10 * WP : 18 * WP].transpose([1, 0, 2]),
            )
            nc.sync.dma_start(
                out=x_sb[:, :, 0, 18 * WP : 34 * WP],
                in_=x8[:, 0, :, 18 * WP : 34 * WP].transpose([1, 0, 2]),
            )
            nc.sync.dma_start(
                out=x_sb[:, :, 0, 34 * WP :], in_=x8[:, 0, :, 34 * WP :].transpose([1, 0, 2])
            )
            nc.gpsimd.dma_start(out=b_sb, in_=br)
            nc.gpsimd.dma_start(out=w_sb[:, 1], in_=wts_v[:, 1])
            for n in range(1, N_IMG):
                nc.gpsimd.dma_start(
                    out=x_sb[:, :, n, :], in_=x8[:, n, :, :].transpose([1, 0, 2])
                )

            # strides for the hand-built rhs views
            hi0 = x_sb[:, 0, 0, :]
            pstride = hi0.ap[0][0]
            d_lo = x_sb[:, 1, 0, :].offset - hi0.offset  # hi -> lo k-tile stride

            out_q = 0  # alternate output stores across both DMA queues
            for n in range(N_IMG):
                base = x_sb[:, 0, n, :]  # hi plane of image n
                for o2 in range(2):
                    o_sb = opool.tile([128, SO], bf16, tag="o_sb")
                    for r in range(RCHUNKS):
                        is_last = n == N_IMG - 1 and o2 == 1 and r == RCHUNKS - 1
                        bias_ap = b_sb[:, o2 : o2 + 1]
                        o_slice = out[n, o2 * 128 : (o2 + 1) * 128, r * NCH : (r + 1) * NCH]

                        def chunk_matmuls(ps, r0, col0, ncol):
                            # (xh + xl) * wh : all 9 taps, hi/lo k-tile pairs
                            for t in range(NTAP):
                                rhs = _sv(
                                    base,
                                    [[pstride, 128], [d_lo, 2], [WP, ncol // W], [1, W]],
                                    extra=r0 + col0 + TAP_OFF[t],
                                )
                                nc.tensor.matmul(
                                    ps,
                                    lhsT=w_sb[:, o2, 2 * t : 2 * t + 2, :],
                                    rhs=rhs,
                                    start=(t == 0),
                                    stop=False,
                                    perf_mode=DR,
                                )
                            # xh * wl : taps (p, p+3) pairs, k-tile stride 58
                            for p in range(3):
                                rhs = _sv(
                                    base,
                                    [
                                        [pstride, 128],
                                        [TAP_OFF[p + 3] - TAP_OFF[p], 2],
                                        [WP, ncol // W],
                                        [1, W],
                                    ],
                                    extra=r0 + col0 + TAP_OFF[p],
                                )
                                nc.tensor.matmul(
                                    ps,
                                    lhsT=w_sb[:, o2, 18 + 2 * p : 18 + 2 * p + 2, :],
                                    rhs=rhs,
                                    start=False,
                                    stop=(p == 2),
                                    perf_mode=DR,
                                )

                        r0 = r * NROW * WP
                        if is_last:
                            # tail chunk: two half-groups (N=224) so draining
                            # starts before the final matmul; copies on both
                            # engines into private tiles (no false deps) and
                            # stores spread over both DMA queues
                            hc = NCH // 2
                            qc = NCH // 4
                            for half in range(2):
                                psh = ppool.tile([128, hc], f32, tag="ps")
                                chunk_matmuls(psh, r0, half * hc // W * WP, hc)
                                for qq in range(2):
                                    q = 2 * half + qq
                                    s_ps = slice(qq * qc, (qq + 1) * qc)
                                    s_out = slice(q * qc, (q + 1) * qc)
                                    t_sb = opool.tile([128, qc], bf16, tag=f"tail{q}")
                                    if qq == 0:
                                        nc.vector.tensor_scalar_add(t_sb, psh[:, s_ps], bias_ap)
                                    else:
                                        nc.scalar.activation(
                                            t_sb,
                                            psh[:, s_ps],
                                            mybir.ActivationFunctionType.Identity,
                                            bias=bias_ap,
                                        )
                                    eng = nc.sync if qq == 0 else nc.gpsimd
                                    eng.dma_start(out=o_slice[:, s_out], in_=t_sb)
                        else:
                            ps = ppool.tile([128, NCH], f32, tag="ps")
                            chunk_matmuls(ps, r0, 0, NCH)
                            dst = o_sb[:, r * NCH : (r + 1) * NCH]
                            if r % 2 == 0:
                                nc.vector.tensor_scalar_add(dst, ps, bias_ap)
                            else:
                                nc.scalar.activation(
                                    dst, ps, mybir.ActivationFunctionType.Identity, bias=bias_ap
                                )
                            eng = nc.sync if out_q % 2 == 0 else nc.gpsimd
                            out_q += 1
                            eng.dma_start(out=o_slice, in_=dst)

    nc.compile()
    return nc


def _get_module():
    if "nc" not in _CACHE:
        _CACHE["nc"] = _build_module()
    return _CACHE["nc"]


def kernel(x, weight, bias):
    x = np.asarray(x, dtype=np.float32)
    weight = np.asarray(weight, dtype=np.float32)
    bias = np.asarray(bias, dtype=np.float32)

    xp = np.pad(x, ((0, 0), (0, 0), (1, 1), (1, 1))).reshape(32, C_IN, SP)
    xh = xp.astype(NP8)
    xl = (xp - xh.astype(np.float32)).astype(NP8)

    # weight (O, I, 3, 3) -> [I, tap, O] fp8 hi + lo
    wt = np.ascontiguousarray(weight.transpose(1, 2, 3, 0)).reshape(C_IN, NTAP, C_OUT)
    wh = wt.astype(NP8)
    wlv = (wt - wh.astype(np.float32)).astype(NP8)
    wh_s = wh.reshape(C_IN, NTAP, 2, 128).transpose(0, 2, 1, 3)  # [c, o2, tap, 128]
    wl_s = wlv.reshape(C_IN, NTAP, 2, 128).transpose(0, 2, 1, 3)
    # wts: [c, o2, slot24, 128]
    wts = np.empty((C_IN, 2, 24, 128), dtype=NP8)
    for t in range(NTAP):
        wts[:, :, 2 * t] = wh_s[:, :, t]
        wts[:, :, 2 * t + 1] = wh_s[:, :, t]
    for p in range(3):
        wts[:, :, 18 + 2 * p] = wl_s[:, :, p]
        wts[:, :, 18 + 2 * p + 1] = wl_s[:, :, p + 3]
    wts = np.ascontiguousarray(wts).reshape(C_IN, -1)
    br = np.ascontiguousarray(bias.reshape(2, 128).T)

    nc = _get_module()
    in_maps = [
        {
            "x8": np.ascontiguousarray(
                np.stack([xh[N_IMG * c : N_IMG * (c + 1)], xl[N_IMG * c : N_IMG * (c + 1)]])
            ),
            "wts": wts,
            "br": br,
        }
        for c in range(N_CORES)
    ]
    res = run_bass_kernel_spmd(nc, in_maps, core_ids=list(range(N_CORES)))
    outs = [
        np.asarray(r["out"]).astype(np.float32).reshape(N_IMG, C_OUT, H, W)
        for r in res.results
    ]
    return np.concatenate(outs, axis=0)
